# revision 40
# baseline (speedup 1.0000x reference)
"""Trainium2 kernel for the NNUE-style factorized embedding segment-sum.

Strategy: the ragged two-table embedding-bag is reformulated as block-diagonal
dense matmuls.  For each output row (bag), the gather+segment-sum over its
ragged feature ids equals  counts_row @ table_block, where table_block is the
768-row slice of the merged factorized table selected by the bag's king square
(and counts columns are flip-remapped for the second output so only ONE table
is ever needed).

Host (integer work only): merge the factor tables (tiles + (pieces+ranks+
files)*mask -> [64, 768, 256]), build per-bag count rows, group (output,bag)
items by table block, shard blocks over 8 cores, clip outputs.  Device (fp
work): per 128-item chunk, 6 accumulating matmuls (K=128, M=128, N=256) and
a PSUM->fp16 drain.

Default mode "mgd8": merged table in fp16, counts as fp8e4 (ints <= 16 exact,
consumed by the matmul directly, no on-device cast), outputs in fp16
(upcast + clip on host).  Fallbacks: "mgdu8" (uint8 counts + on-device cast)
if counts exceed 16, and the original factorized "hilo"/"f32r" paths.

Scheduling (from NTFF profiling; 82us -> ~43us): table loads ride the ACT
HWDGE ring (first two up front split fine, the rest two slots ahead so they
do not starve count loads of shared SDMA bandwidth during fill), count loads
ride the SP ring, batched per-slot stores ride the ACT ring behind all table
loads; dummy matmuls on memset tiles warm the PE p-state (0.65->2.4 GHz over
~3us) during the DMA fill; the final store is kept small because its HBM
write receipt gates the exit barrier.

Blocks are assigned to (core, slot) so that each slot's chunk capacity (shared
across cores — the compiled program is SPMD) matches the data tightly; for
this input the packing is optimal (36 chunks/core vs 35.875 ideal).
"""

import numpy as np
import ml_dtypes

import concourse.bass as bass
import concourse.tile as tile
from concourse import bacc, mybir
from concourse.bass_utils import run_bass_kernel_spmd

N_CORES = 8
B = 16384          # bags
KPL = 12           # piece planes
DOUT = 256
PIECE = 768        # KPL * 64
NFEAT = 972        # 768 tiles + 12 pieces + 96 ranks + 96 files (factorized)
NBLK = 8           # table blocks per core (64 king squares / 8 cores)

# ---------------------------------------------------------------------------
# host-side integer prep tables
_sq = np.arange(64)
_PERM = (7 - _sq // 8) * 8 + _sq % 8          # vertical king-square flip
_v = np.arange(PIECE)
_vk, _vr, _vf = _v // 64, (_v % 64) // 8, _v % 8
_FLIP_COL = ((_vk + 6) % 12) * 64 + (7 - _vr) * 8 + _vf

_prog_cache = {}


def _mode_params(mode):
    f32 = mybir.dt.float32
    if mode == "mgd4":
        # merged fp16 table, nibble-packed counts (two 4-bit counts per
        # byte) unpacked on DVE/Pool, fp16 out (clip on host)
        return dict(nchk=6, npass=1, tdt=mybir.dt.float16,
                    cdt=mybir.dt.uint8, mdt=mybir.dt.float16,
                    odt=mybir.dt.float16, ccols=384)
    if mode == "mgd8":
        # merged fp16 table, fp8e4 counts straight into the matmul, fp16 out
        return dict(nchk=6, npass=1, tdt=mybir.dt.float16,
                    cdt=mybir.dt.float8e4, mdt=mybir.dt.float8e4,
                    odt=mybir.dt.float16, ccols=768)
    if mode == "mgdu8":
        return dict(nchk=6, npass=1, tdt=mybir.dt.float16,
                    cdt=mybir.dt.uint8, mdt=mybir.dt.float16,
                    odt=mybir.dt.float16, ccols=768)
    if mode == "hilo":
        return dict(nchk=8, npass=2, tdt=mybir.dt.bfloat16,
                    cdt=mybir.dt.uint8, mdt=mybir.dt.bfloat16, odt=f32,
                    ccols=1024)
    # f32r: factorized, fp32 tables with reduced-precision matmul
    return dict(nchk=8, npass=1, tdt=mybir.dt.float32r,
                cdt=mybir.dt.float32r, mdt=mybir.dt.float32r, odt=f32,
                ccols=1024)


def _build_program(caps: tuple, mode: str):
    """Bass program for one core.

    caps[s] = number of 128-item chunks for block slot s (shared by all
    cores).  Per slot: DMA table block + counts, (maybe) cast counts, then per
    chunk npass*nchk accumulating matmuls and a clipped PSUM->SBUF->HBM drain.
    """
    p = _mode_params(mode)
    nchk, npass = p["nchk"], p["npass"]
    tdt, cdt, mdt, odt = p["tdt"], p["cdt"], p["mdt"], p["odt"]
    ccols = p["ccols"]
    nib = mode == "mgd4"
    cast = cdt != mdt and not nib

    nch = sum(caps)
    nc = bacc.Bacc("TRN2", target_bir_lowering=False, debug=False)
    f32 = mybir.dt.float32

    tabw = npass * nchk * DOUT
    # tab[p, blk*tabw + (pass*nchk+j)*DOUT + d] = table[blk,pass][j*128+p, d]
    tab = nc.dram_tensor("tab", [128, NBLK * tabw], tdt,
                         kind="ExternalInput").ap()
    # cm[p, (chunkbase(s)+i)*ccols + j*128 + m]
    #    = counts^T[slot s, chunk i][feature j*128+p, item m]
    # (mgd4: byte packs features f and f+384: lo nibble f, hi nibble f+384)
    cm = nc.dram_tensor("cm", [128, nch * ccols], cdt,
                        kind="ExternalInput").ap()
    # out[p, (chunkbase(s)+i)*DOUT + d]: partition-major so each per-slot
    # store is one DMA with caps*512B contiguous per partition line
    out = nc.dram_tensor("out", [128, nch * DOUT], odt,
                         kind="ExternalOutput").ap()

    cbase = np.concatenate([[0], np.cumsum(caps)]).astype(int)
    maxw = max(caps) * ccols

    with tile.TileContext(nc) as tc:
        with (
            tc.tile_pool(name="tabp", bufs=NBLK) as tabp,
            tc.tile_pool(name="cmup", bufs=8) as cmup,
            tc.tile_pool(name="cmp", bufs=5) as cmp_,
            tc.tile_pool(name="outp", bufs=8) as outp,
            tc.tile_pool(name="warmp", bufs=1) as wmp,
            tc.tile_pool(name="ps", bufs=8, space="PSUM") as psp,
        ):
            # PE p-state warmup: the tensor engine ramps 0.65->1.2->2.4 GHz
            # over ~3us of continuous execution.  Run dummy matmuls on
            # memset tiles during the DMA fill window so the real matmul
            # stream starts at full clock.
            wl = wmp.tile([128, 128], mdt, tag="warml")
            wr = wmp.tile([128, DOUT], tdt, tag="warmr")
            nc.gpsimd.memset(wl[:], 0)
            nc.gpsimd.memset(wr[:], 0)
            wp = psp.tile([128, DOUT], f32, tag="ps")
            for _ in range(8):
                nc.tensor.matmul(wp[:], lhsT=wl[:], rhs=wr[:])

            # table loads go on the ACT HWDGE ring (stores are emitted
            # later, so prefetches never block behind them).  Only the
            # first two tables load up front: both rings share the 16 SDMA
            # engines round-robin, so eagerly loading all tables would
            # halve the bandwidth available to the count loads during the
            # pipeline fill.  The rest are issued two slots ahead.
            def load_tab(b, tsplit=1):
                tt = tabp.tile([128, tabw], tdt, tag="tab")
                tb = [tabw * k // tsplit // DOUT * DOUT
                      for k in range(tsplit + 1)]
                for k in range(tsplit):
                    nc.scalar.dma_start(
                        tt[:, tb[k]:tb[k + 1]],
                        tab[:, b * tabw + tb[k]:b * tabw + tb[k + 1]])
                return tt

            tts = [load_tab(0, tsplit=2), load_tab(1)]
            for b in range(NBLK):
                cmw = caps[b] * ccols
                c0 = cbase[b] * ccols
                # split ranges: slot 0 goes [chunk0, chunk1, rest] — the
                # first two matmul chunks get their own completion sems so
                # the stream starts early, without paying per-chunk issue
                # cost (~650ns each) that would delay the cm loads of the
                # following slots.  Whole-slot afterwards.
                if b == 0:
                    bnds = sorted(set([0, ccols, min(2 * ccols, cmw), cmw]))
                    nsplit = len(bnds) - 1
                else:
                    bnds = [0, cmw]
                    nsplit = 1
                tt = tts[b]
                cu = cmup.tile([128, maxw], cdt, tag="cmu")
                for k in range(nsplit):
                    nc.sync.dma_start(
                        cu[:, bnds[k]:bnds[k + 1]],
                        cm[:, c0 + bnds[k]:c0 + bnds[k + 1]])
                if b + 2 < NBLK:
                    tts.append(load_tab(b + 2))
                if nib:
                    # unpack nibbles: lo -> feature chunks 0..2, hi -> 3..5;
                    # lo on DVE, hi on Pool (gpsimd)
                    h0 = caps[b] * 384
                    cmt = cmp_.tile([128, 2 * maxw], mdt, tag="cm")
                    for k in range(nsplit):
                        nc.vector.tensor_scalar(
                            cmt[:, bnds[k]:bnds[k + 1]],
                            cu[:, bnds[k]:bnds[k + 1]],
                            15, None, mybir.AluOpType.bitwise_and)
                        nc.gpsimd.tensor_scalar(
                            cmt[:, h0 + bnds[k]:h0 + bnds[k + 1]],
                            cu[:, bnds[k]:bnds[k + 1]],
                            4, None, mybir.AluOpType.logical_shift_right)
                elif cast:
                    cmt = cmp_.tile([128, maxw], mdt, tag="cm")
                    # 8-bit -> 16-bit cast, split so it pipelines; alternate
                    # DVE / Pool so neither engine becomes the bottleneck
                    ncast = max(nsplit, 2)
                    cbnds = [cmw * k // ncast // 128 * 128
                             for k in range(ncast + 1)]
                    for k in range(ncast):
                        eng = nc.vector if k % 2 == 0 else nc.gpsimd
                        eng.tensor_copy(cmt[:, cbnds[k]:cbnds[k + 1]],
                                        cu[:, cbnds[k]:cbnds[k + 1]])
                else:
                    cmt = cu

                outt = outp.tile([128, caps[b] * DOUT], odt, tag="out")
                for i in range(caps[b]):
                    ps = psp.tile([128, DOUT], f32, tag="ps")
                    nmm = npass * nchk
                    for q in range(nmm):
                        p_, j = divmod(q, nchk)
                        if nib:
                            cb_ = (j // 3) * h0 + (i * 3 + j % 3) * 128
                        else:
                            cb_ = (i * nchk + j) * 128
                        lhsT = cmt[:, cb_:cb_ + 128]
                        nc.tensor.matmul(
                            ps[:],
                            lhsT=lhsT,
                            rhs=tt[:, (p_ * nchk + j) * DOUT:
                                   (p_ * nchk + j + 1) * DOUT],
                            start=(q == 0),
                            stop=(q == nmm - 1),
                        )
                    # clip(psum, 0, 1) -> per-slot sbuf tile (per chunk)
                    nc.vector.tensor_scalar(
                        outt[:, i * DOUT:(i + 1) * DOUT], ps[:],
                        1.0, 0.0, mybir.AluOpType.min, mybir.AluOpType.max)
                if b < NBLK - 1:
                    # one batched store per slot on the ACT HWDGE ring
                    nc.scalar.dma_start(
                        out[:, cbase[b] * DOUT:(cbase[b] + caps[b]) * DOUT],
                        outt[:])
                else:
                    # last slot: all-but-last chunks in one store, then the
                    # final chunk alone so the last HBM write receipt (which
                    # gates the exit barrier) covers a small transfer
                    if caps[b] > 1:
                        nc.scalar.dma_start(
                            out[:, cbase[b] * DOUT:
                                (cbase[b] + caps[b] - 1) * DOUT],
                            outt[:, :(caps[b] - 1) * DOUT])
                    nc.scalar.dma_start(
                        out[:, (cbase[b] + caps[b] - 1) * DOUT:
                            (cbase[b] + caps[b]) * DOUT],
                        outt[:, (caps[b] - 1) * DOUT:])

    nc.compile()
    return nc


def _prep(values, lengths, kings, mask, merged):
    """Host prep: counts, per-core item layout; factor sums if not merged."""
    values = np.asarray(values).astype(np.int64)
    lengths = np.asarray(lengths).astype(np.int64)
    kings = np.asarray(kings).astype(np.int64)
    maskrows = np.asarray(mask, np.float32).reshape(64, PIECE)

    seg = np.repeat(np.arange(B, dtype=np.int64), lengths)

    # counts in merged-table column space; output b columns are flip-remapped
    cnt_a = np.bincount(seg * PIECE + values,
                        minlength=B * PIECE).reshape(B, PIECE)
    cnt_b = np.bincount(seg * PIECE + _FLIP_COL[values],
                        minlength=B * PIECE).reshape(B, PIECE)

    # block id per (output,bag) item, in merged-table space
    blk = np.concatenate([kings[:, 0], _PERM[kings[:, 1]]])

    nfp = PIECE if merged else 1024
    ext = np.zeros((2 * B + 1, nfp), np.float32)  # last row stays zero (pad)
    cnt = ext[:2 * B, :PIECE]
    cnt[:B] = cnt_a
    cnt[B:] = cnt_b
    cmax = float(cnt.max())
    if not merged:
        # factorized extension: mask-weighted per-(k), (k,rank), (k,file) sums
        m = (cnt * maskrows[blk]).reshape(2 * B, KPL, 8, 8)
        ext[:2 * B, PIECE:PIECE + KPL] = m.sum(axis=(2, 3))
        ext[:2 * B, PIECE + KPL:PIECE + KPL + 96] = \
            m.sum(axis=3).reshape(2 * B, 96)
        ext[:2 * B, PIECE + KPL + 96:NFEAT] = \
            m.sum(axis=2).reshape(2 * B, 96)

    order = np.argsort(blk, kind="stable")
    nper = np.bincount(blk, minlength=64)
    offs = np.concatenate([[0], np.cumsum(nper)])
    nchunks = np.maximum(np.ceil(nper / 128).astype(int), 1)

    # assign blocks to (core, slot): sort by descending chunk need so each
    # slot's shared capacity is tight
    rank = np.argsort(-nchunks, kind="stable")      # block ids, desc need
    caps = tuple(int(nchunks[rank[s * N_CORES]]) for s in range(NBLK))
    cbase = np.concatenate([[0], np.cumsum(caps)]).astype(int)
    nch = int(cbase[-1])

    pad_idx = np.full((N_CORES, nch * 128), -1, np.int64)
    for s in range(NBLK):
        for c in range(N_CORES):
            t = rank[s * N_CORES + c]               # block for (core c, slot s)
            ids = order[offs[t]:offs[t + 1]]
            base = cbase[s] * 128
            pad_idx[c, base:base + len(ids)] = ids

    # block index (0..63) per (core, slot), for table selection
    blk_of = rank.reshape(NBLK, N_CORES).T          # [core, slot]

    u8_ok = (np.all((maskrows == 0.0) | (maskrows == 1.0))
             and ext.max() <= 255.0)
    return ext, pad_idx, caps, blk_of, cmax, u8_ok


def _make_tab_merged(pieces, ranks, files, tiles, mask, blk_of, tdt_np):
    """Per-core [128, NBLK*6*DOUT]: host-merged factorized table."""
    p = np.asarray(pieces, np.float32)   # [64,12,1,1,256]
    r = np.asarray(ranks, np.float32)    # [64,12,8,1,256]
    f = np.asarray(files, np.float32)    # [64,12,1,8,256]
    t = np.asarray(tiles, np.float32)    # [64,12,8,8,256]
    m = np.asarray(mask, np.float32)     # [64,12,8,8,1]
    merged = (t + (p + r + f) * m).reshape(64, PIECE, DOUT).astype(tdt_np)
    planes = merged.reshape(64, 6, 128, DOUT)
    tabs = []
    for c in range(N_CORES):
        tc_ = planes[blk_of[c]]                # [8, 6, 128, 256]
        tabs.append(np.ascontiguousarray(
            tc_.transpose(2, 0, 1, 3).reshape(128, -1)))
    return tabs


def _make_tab_fact(pieces, ranks, files, tiles, blk_of, mode):
    """Per-core factorized tables (hilo / f32r fallback paths)."""
    pieces = np.asarray(pieces, np.float32).reshape(64, KPL, DOUT)
    ranks = np.asarray(ranks, np.float32).reshape(64, KPL * 8, DOUT)
    files = np.asarray(files, np.float32).reshape(64, KPL * 8, DOUT)
    tiles = np.asarray(tiles, np.float32).reshape(64, PIECE, DOUT)
    big = np.zeros((64, 1024, DOUT), np.float32)
    big[:, :PIECE] = tiles
    big[:, PIECE:PIECE + KPL] = pieces
    big[:, PIECE + KPL:PIECE + KPL + 96] = ranks
    big[:, PIECE + KPL + 96:NFEAT] = files

    bf16 = ml_dtypes.bfloat16
    if mode == "hilo":
        hi = big.astype(bf16)
        lo = (big - hi.astype(np.float32)).astype(bf16)
        planes = np.stack([hi, lo], axis=1).reshape(64, 2, 8, 128, DOUT)
    else:
        planes = big.reshape(64, 1, 8, 128, DOUT)

    tabs = []
    for c in range(N_CORES):
        t = planes[blk_of[c]]                  # [8, npass, 8, 128, DOUT]
        t = t.transpose(3, 0, 1, 2, 4)         # [128, slot, pass, chunk, dout]
        tabs.append(np.ascontiguousarray(t.reshape(128, -1)))
    return tabs


def _run(inputs, trace=False, force_mode=None):
    merged_first = force_mode is None or force_mode.startswith("mgd")
    ext, pad_idx, caps, blk_of, cmax, u8_ok = _prep(
        inputs["values"], inputs["lengths"], inputs["kings"],
        inputs["factorization_mask"], merged=merged_first)
    if force_mode:
        mode = force_mode
    elif cmax <= 16.0:       # ints <= 16 are exact in fp8 e4m3
        mode = "mgd8"
    elif cmax <= 255.0:
        mode = "mgdu8"
    else:
        mode = "f32r"
    if merged_first and not mode.startswith("mgd"):
        ext, pad_idx, caps, blk_of, cmax, u8_ok = _prep(
            inputs["values"], inputs["lengths"], inputs["kings"],
            inputs["factorization_mask"], merged=False)
    p = _mode_params(mode)
    nchk = p["nchk"]
    cm_np = np.dtype(mybir.dt.np(p["cdt"]))
    out_np = np.dtype(mybir.dt.np(p["odt"]))
    tdt_np = np.dtype(mybir.dt.np(p["tdt"] if p["tdt"] != mybir.dt.float32r
                                  else mybir.dt.float32))

    nch = sum(caps)
    key = (caps, mode)
    if key not in _prog_cache:
        _prog_cache[key] = _build_program(caps, mode)
    nc = _prog_cache[key]

    if mode.startswith("mgd"):
        tabs = _make_tab_merged(inputs["pieces"], inputs["ranks"],
                                inputs["files"], inputs["tiles"],
                                inputs["factorization_mask"], blk_of, tdt_np)
    else:
        tabs = _make_tab_fact(inputs["pieces"], inputs["ranks"],
                              inputs["files"], inputs["tiles"], blk_of, mode)

    in_maps = []
    for c in range(N_CORES):
        sel = ext[pad_idx[c]]                  # [nch*128, nfp] f32
        cmh = sel.reshape(nch, 128, nchk, 128).transpose(3, 0, 2, 1)
        if mode == "mgd4":
            # nibble-pack: byte = count[f] | count[f+384] << 4
            cmh = cmh[:, :, :3, :] + 16.0 * cmh[:, :, 3:, :]
        in_maps.append({
            "tab": tabs[c],
            "cm": np.ascontiguousarray(cmh.reshape(128, -1).astype(cm_np)),
        })

    res = run_bass_kernel_spmd(nc, in_maps, list(range(N_CORES)),
                               trace=trace)

    comb = np.zeros((2 * B, DOUT), np.float32)
    for c in range(N_CORES):
        # out dram layout is [128, nch*DOUT] partition-major; the device
        # stores raw sums — the clip to [0, 1] happens here on the host
        flat = (res.results[c]["out"].astype(np.float32)
                .reshape(128, nch, DOUT).transpose(1, 0, 2)
                .reshape(nch * 128, DOUT))
        valid = pad_idx[c] >= 0
        comb[pad_idx[c][valid]] = flat[valid]
    np.clip(comb, 0.0, 1.0, out=comb)
    return (comb[:B], comb[B:]), res


def kernel(**inputs):
    (a, b), _ = _run(inputs, trace=False)
    return a, b


# revision 41
# speedup vs baseline: 1.1346x; 1.1346x over previous
"""Trainium2 kernel for the NNUE-style factorized embedding segment-sum.

Strategy: the ragged two-table embedding-bag is reformulated as block-diagonal
dense matmuls.  For each output row (bag), the gather+segment-sum over its
ragged feature ids equals  counts_row @ table_block, where table_block is the
768-row slice of the merged factorized table selected by the bag's king square
(and counts columns are flip-remapped for the second output so only ONE table
is ever needed).

Host (integer work only): merge the factor tables (tiles + (pieces+ranks+
files)*mask -> [64, 768, 256]), build per-bag count rows, group (output,bag)
items by table block, shard blocks over 8 cores, clip outputs.  Device (fp
work): per 128-item chunk, 6 accumulating matmuls (K=128, M=128, N=256) and
a PSUM->fp16 drain.

Default mode "mgd8": merged table in fp16, counts as fp8e4 (ints <= 16 exact,
consumed by the matmul directly, no on-device cast), outputs in fp16
(upcast + clip on host).  Fallbacks: "mgdu8" (uint8 counts + on-device cast)
if counts exceed 16, and the original factorized "hilo"/"f32r" paths.

Scheduling (from NTFF profiling; 82us -> ~43us): table loads ride the ACT
HWDGE ring (first two up front split fine, the rest two slots ahead so they
do not starve count loads of shared SDMA bandwidth during fill), count loads
ride the SP ring, batched per-slot stores ride the ACT ring behind all table
loads; dummy matmuls on memset tiles warm the PE p-state (0.65->2.4 GHz over
~3us) during the DMA fill; the final store is kept small because its HBM
write receipt gates the exit barrier.

Blocks are assigned to (core, slot) so that each slot's chunk capacity (shared
across cores — the compiled program is SPMD) matches the data tightly; for
this input the packing is optimal (36 chunks/core vs 35.875 ideal).
"""

import numpy as np
import ml_dtypes

import concourse.bass as bass
import concourse.tile as tile
from concourse import bacc, mybir
from concourse.bass_utils import run_bass_kernel_spmd

N_CORES = 8
B = 16384          # bags
KPL = 12           # piece planes
DOUT = 256
PIECE = 768        # KPL * 64
NFEAT = 972        # 768 tiles + 12 pieces + 96 ranks + 96 files (factorized)
NBLK = 8           # table blocks per core (64 king squares / 8 cores)

# ---------------------------------------------------------------------------
# host-side integer prep tables
_sq = np.arange(64)
_PERM = (7 - _sq // 8) * 8 + _sq % 8          # vertical king-square flip
_v = np.arange(PIECE)
_vk, _vr, _vf = _v // 64, (_v % 64) // 8, _v % 8
_FLIP_COL = ((_vk + 6) % 12) * 64 + (7 - _vr) * 8 + _vf

_prog_cache = {}


def _mode_params(mode):
    f32 = mybir.dt.float32
    if mode == "mgd4":
        # merged fp16 table, nibble-packed counts (two 4-bit counts per
        # byte) unpacked on DVE/Pool, fp16 out (clip on host)
        return dict(nchk=6, npass=1, tdt=mybir.dt.float16,
                    cdt=mybir.dt.uint8, mdt=mybir.dt.float16,
                    odt=mybir.dt.float16, ccols=384)
    if mode == "mgd8":
        # merged fp16 table, fp8e4 counts straight into the matmul, fp16 out
        return dict(nchk=6, npass=1, tdt=mybir.dt.float16,
                    cdt=mybir.dt.float8e4, mdt=mybir.dt.float8e4,
                    odt=mybir.dt.float16, ccols=768)
    if mode == "mgdu8":
        return dict(nchk=6, npass=1, tdt=mybir.dt.float16,
                    cdt=mybir.dt.uint8, mdt=mybir.dt.float16,
                    odt=mybir.dt.float16, ccols=768)
    if mode == "hilo":
        return dict(nchk=8, npass=2, tdt=mybir.dt.bfloat16,
                    cdt=mybir.dt.uint8, mdt=mybir.dt.bfloat16, odt=f32,
                    ccols=1024)
    # f32r: factorized, fp32 tables with reduced-precision matmul
    return dict(nchk=8, npass=1, tdt=mybir.dt.float32r,
                cdt=mybir.dt.float32r, mdt=mybir.dt.float32r, odt=f32,
                ccols=1024)


def _build_program(caps: tuple, mode: str):
    """Bass program for one core.

    caps[s] = number of 128-item chunks for block slot s (shared by all
    cores).  Per slot: DMA table block + counts, (maybe) cast counts, then per
    chunk npass*nchk accumulating matmuls and a clipped PSUM->SBUF->HBM drain.
    """
    p = _mode_params(mode)
    nchk, npass = p["nchk"], p["npass"]
    tdt, cdt, mdt, odt = p["tdt"], p["cdt"], p["mdt"], p["odt"]
    ccols = p["ccols"]
    nib = mode == "mgd4"
    cast = cdt != mdt and not nib

    nch = sum(caps)
    nc = bacc.Bacc("TRN2", target_bir_lowering=False, debug=False)
    f32 = mybir.dt.float32

    tabw = npass * nchk * DOUT
    # tab[p, blk*tabw + (pass*nchk+j)*DOUT + d] = table[blk,pass][j*128+p, d]
    tab = nc.dram_tensor("tab", [128, NBLK * tabw], tdt,
                         kind="ExternalInput").ap()
    # cm[p, (chunkbase(s)+i)*ccols + j*128 + m]
    #    = counts^T[slot s, chunk i][feature j*128+p, item m]
    # (mgd4: byte packs features f and f+384: lo nibble f, hi nibble f+384)
    cm = nc.dram_tensor("cm", [128, nch * ccols], cdt,
                        kind="ExternalInput").ap()
    # out[p, (chunkbase(s)+i)*DOUT + d]: partition-major so each per-slot
    # store is one DMA with caps*512B contiguous per partition line
    out = nc.dram_tensor("out", [128, nch * DOUT], odt,
                         kind="ExternalOutput").ap()

    cbase = np.concatenate([[0], np.cumsum(caps)]).astype(int)
    maxw = max(caps) * ccols

    with tile.TileContext(nc) as tc:
        with (
            tc.tile_pool(name="tabp", bufs=NBLK) as tabp,
            tc.tile_pool(name="cmup", bufs=8) as cmup,
            tc.tile_pool(name="cmp", bufs=5) as cmp_,
            tc.tile_pool(name="outp", bufs=8) as outp,
            tc.tile_pool(name="warmp", bufs=1) as wmp,
            tc.tile_pool(name="ps", bufs=8, space="PSUM") as psp,
        ):
            # PE p-state warmup: the tensor engine ramps 0.65->1.2->2.4 GHz
            # over ~3us of continuous execution.  Run dummy matmuls on
            # memset tiles during the DMA fill window so the real matmul
            # stream starts at full clock.
            wl = wmp.tile([128, 128], mdt, tag="warml")
            wr = wmp.tile([128, DOUT], tdt, tag="warmr")
            nc.gpsimd.memset(wl[:], 0)
            nc.gpsimd.memset(wr[:], 0)
            wp = psp.tile([128, DOUT], f32, tag="ps")
            for _ in range(10):
                nc.tensor.matmul(wp[:], lhsT=wl[:], rhs=wr[:])

            # table loads go on the ACT HWDGE ring (stores are emitted
            # later, so prefetches never block behind them).  Only the
            # first two tables load up front: both rings share the 16 SDMA
            # engines round-robin, so eagerly loading all tables would
            # halve the bandwidth available to the count loads during the
            # pipeline fill.  The rest are issued two slots ahead.
            def load_tab(b, tsplit=1):
                tt = tabp.tile([128, tabw], tdt, tag="tab")
                tb = [tabw * k // tsplit // DOUT * DOUT
                      for k in range(tsplit + 1)]
                for k in range(tsplit):
                    nc.scalar.dma_start(
                        tt[:, tb[k]:tb[k + 1]],
                        tab[:, b * tabw + tb[k]:b * tabw + tb[k + 1]])
                return tt

            tts = [load_tab(0, tsplit=3), load_tab(1)]
            for b in range(NBLK):
                cmw = caps[b] * ccols
                c0 = cbase[b] * ccols
                # split ranges: slot 0 goes [chunk0, chunk1, rest] — the
                # first two matmul chunks get their own completion sems so
                # the stream starts early, without paying per-chunk issue
                # cost (~650ns each) that would delay the cm loads of the
                # following slots.  Whole-slot afterwards.
                if b == 0:
                    bnds = sorted(set([0, ccols, min(2 * ccols, cmw), cmw]))
                    nsplit = len(bnds) - 1
                else:
                    bnds = [0, cmw]
                    nsplit = 1
                tt = tts[b]
                cu = cmup.tile([128, maxw], cdt, tag="cmu")
                for k in range(nsplit):
                    nc.sync.dma_start(
                        cu[:, bnds[k]:bnds[k + 1]],
                        cm[:, c0 + bnds[k]:c0 + bnds[k + 1]])
                if b + 2 < NBLK:
                    tts.append(load_tab(b + 2))
                if nib:
                    # unpack nibbles: lo -> feature chunks 0..2, hi -> 3..5;
                    # lo on DVE, hi on Pool (gpsimd)
                    h0 = caps[b] * 384
                    cmt = cmp_.tile([128, 2 * maxw], mdt, tag="cm")
                    for k in range(nsplit):
                        nc.vector.tensor_scalar(
                            cmt[:, bnds[k]:bnds[k + 1]],
                            cu[:, bnds[k]:bnds[k + 1]],
                            15, None, mybir.AluOpType.bitwise_and)
                        nc.gpsimd.tensor_scalar(
                            cmt[:, h0 + bnds[k]:h0 + bnds[k + 1]],
                            cu[:, bnds[k]:bnds[k + 1]],
                            4, None, mybir.AluOpType.logical_shift_right)
                elif cast:
                    cmt = cmp_.tile([128, maxw], mdt, tag="cm")
                    # 8-bit -> 16-bit cast, split so it pipelines; alternate
                    # DVE / Pool so neither engine becomes the bottleneck
                    ncast = max(nsplit, 2)
                    cbnds = [cmw * k // ncast // 128 * 128
                             for k in range(ncast + 1)]
                    for k in range(ncast):
                        eng = nc.vector if k % 2 == 0 else nc.gpsimd
                        eng.tensor_copy(cmt[:, cbnds[k]:cbnds[k + 1]],
                                        cu[:, cbnds[k]:cbnds[k + 1]])
                else:
                    cmt = cu

                outt = outp.tile([128, caps[b] * DOUT], odt, tag="out")
                for i in range(caps[b]):
                    ps = psp.tile([128, DOUT], f32, tag="ps")
                    nmm = npass * nchk
                    for q in range(nmm):
                        p_, j = divmod(q, nchk)
                        if nib:
                            cb_ = (j // 3) * h0 + (i * 3 + j % 3) * 128
                        else:
                            cb_ = (i * nchk + j) * 128
                        lhsT = cmt[:, cb_:cb_ + 128]
                        nc.tensor.matmul(
                            ps[:],
                            lhsT=lhsT,
                            rhs=tt[:, (p_ * nchk + j) * DOUT:
                                   (p_ * nchk + j + 1) * DOUT],
                            start=(q == 0),
                            stop=(q == nmm - 1),
                        )
                    # clip(psum, 0, 1) -> per-slot sbuf tile (per chunk)
                    nc.vector.tensor_scalar(
                        outt[:, i * DOUT:(i + 1) * DOUT], ps[:],
                        1.0, 0.0, mybir.AluOpType.min, mybir.AluOpType.max)
                if b < NBLK - 1:
                    # one batched store per slot on the ACT HWDGE ring
                    nc.scalar.dma_start(
                        out[:, cbase[b] * DOUT:(cbase[b] + caps[b]) * DOUT],
                        outt[:])
                else:
                    # last slot: all-but-last chunks in one store, then the
                    # final chunk alone so the last HBM write receipt (which
                    # gates the exit barrier) covers a small transfer
                    if caps[b] > 1:
                        nc.scalar.dma_start(
                            out[:, cbase[b] * DOUT:
                                (cbase[b] + caps[b] - 1) * DOUT],
                            outt[:, :(caps[b] - 1) * DOUT])
                    nc.scalar.dma_start(
                        out[:, (cbase[b] + caps[b] - 1) * DOUT:
                            (cbase[b] + caps[b]) * DOUT],
                        outt[:, (caps[b] - 1) * DOUT:])

    nc.compile()
    return nc


def _prep(values, lengths, kings, mask, merged):
    """Host prep: counts, per-core item layout; factor sums if not merged."""
    values = np.asarray(values).astype(np.int64)
    lengths = np.asarray(lengths).astype(np.int64)
    kings = np.asarray(kings).astype(np.int64)
    maskrows = np.asarray(mask, np.float32).reshape(64, PIECE)

    seg = np.repeat(np.arange(B, dtype=np.int64), lengths)

    # counts in merged-table column space; output b columns are flip-remapped
    cnt_a = np.bincount(seg * PIECE + values,
                        minlength=B * PIECE).reshape(B, PIECE)
    cnt_b = np.bincount(seg * PIECE + _FLIP_COL[values],
                        minlength=B * PIECE).reshape(B, PIECE)

    # block id per (output,bag) item, in merged-table space
    blk = np.concatenate([kings[:, 0], _PERM[kings[:, 1]]])

    nfp = PIECE if merged else 1024
    ext = np.zeros((2 * B + 1, nfp), np.float32)  # last row stays zero (pad)
    cnt = ext[:2 * B, :PIECE]
    cnt[:B] = cnt_a
    cnt[B:] = cnt_b
    cmax = float(cnt.max())
    if not merged:
        # factorized extension: mask-weighted per-(k), (k,rank), (k,file) sums
        m = (cnt * maskrows[blk]).reshape(2 * B, KPL, 8, 8)
        ext[:2 * B, PIECE:PIECE + KPL] = m.sum(axis=(2, 3))
        ext[:2 * B, PIECE + KPL:PIECE + KPL + 96] = \
            m.sum(axis=3).reshape(2 * B, 96)
        ext[:2 * B, PIECE + KPL + 96:NFEAT] = \
            m.sum(axis=2).reshape(2 * B, 96)

    order = np.argsort(blk, kind="stable")
    nper = np.bincount(blk, minlength=64)
    offs = np.concatenate([[0], np.cumsum(nper)])
    nchunks = np.maximum(np.ceil(nper / 128).astype(int), 1)

    # assign blocks to (core, slot): sort by descending chunk need so each
    # slot's shared capacity is tight
    rank = np.argsort(-nchunks, kind="stable")      # block ids, desc need
    caps = tuple(int(nchunks[rank[s * N_CORES]]) for s in range(NBLK))
    cbase = np.concatenate([[0], np.cumsum(caps)]).astype(int)
    nch = int(cbase[-1])

    pad_idx = np.full((N_CORES, nch * 128), -1, np.int64)
    for s in range(NBLK):
        for c in range(N_CORES):
            t = rank[s * N_CORES + c]               # block for (core c, slot s)
            ids = order[offs[t]:offs[t + 1]]
            base = cbase[s] * 128
            pad_idx[c, base:base + len(ids)] = ids

    # block index (0..63) per (core, slot), for table selection
    blk_of = rank.reshape(NBLK, N_CORES).T          # [core, slot]

    u8_ok = (np.all((maskrows == 0.0) | (maskrows == 1.0))
             and ext.max() <= 255.0)
    return ext, pad_idx, caps, blk_of, cmax, u8_ok


def _make_tab_merged(pieces, ranks, files, tiles, mask, blk_of, tdt_np):
    """Per-core [128, NBLK*6*DOUT]: host-merged factorized table."""
    p = np.asarray(pieces, np.float32)   # [64,12,1,1,256]
    r = np.asarray(ranks, np.float32)    # [64,12,8,1,256]
    f = np.asarray(files, np.float32)    # [64,12,1,8,256]
    t = np.asarray(tiles, np.float32)    # [64,12,8,8,256]
    m = np.asarray(mask, np.float32)     # [64,12,8,8,1]
    merged = (t + (p + r + f) * m).reshape(64, PIECE, DOUT).astype(tdt_np)
    planes = merged.reshape(64, 6, 128, DOUT)
    tabs = []
    for c in range(N_CORES):
        tc_ = planes[blk_of[c]]                # [8, 6, 128, 256]
        tabs.append(np.ascontiguousarray(
            tc_.transpose(2, 0, 1, 3).reshape(128, -1)))
    return tabs


def _make_tab_fact(pieces, ranks, files, tiles, blk_of, mode):
    """Per-core factorized tables (hilo / f32r fallback paths)."""
    pieces = np.asarray(pieces, np.float32).reshape(64, KPL, DOUT)
    ranks = np.asarray(ranks, np.float32).reshape(64, KPL * 8, DOUT)
    files = np.asarray(files, np.float32).reshape(64, KPL * 8, DOUT)
    tiles = np.asarray(tiles, np.float32).reshape(64, PIECE, DOUT)
    big = np.zeros((64, 1024, DOUT), np.float32)
    big[:, :PIECE] = tiles
    big[:, PIECE:PIECE + KPL] = pieces
    big[:, PIECE + KPL:PIECE + KPL + 96] = ranks
    big[:, PIECE + KPL + 96:NFEAT] = files

    bf16 = ml_dtypes.bfloat16
    if mode == "hilo":
        hi = big.astype(bf16)
        lo = (big - hi.astype(np.float32)).astype(bf16)
        planes = np.stack([hi, lo], axis=1).reshape(64, 2, 8, 128, DOUT)
    else:
        planes = big.reshape(64, 1, 8, 128, DOUT)

    tabs = []
    for c in range(N_CORES):
        t = planes[blk_of[c]]                  # [8, npass, 8, 128, DOUT]
        t = t.transpose(3, 0, 1, 2, 4)         # [128, slot, pass, chunk, dout]
        tabs.append(np.ascontiguousarray(t.reshape(128, -1)))
    return tabs


def _run(inputs, trace=False, force_mode=None):
    merged_first = force_mode is None or force_mode.startswith("mgd")
    ext, pad_idx, caps, blk_of, cmax, u8_ok = _prep(
        inputs["values"], inputs["lengths"], inputs["kings"],
        inputs["factorization_mask"], merged=merged_first)
    if force_mode:
        mode = force_mode
    elif cmax <= 16.0:       # ints <= 16 are exact in fp8 e4m3
        mode = "mgd8"
    elif cmax <= 255.0:
        mode = "mgdu8"
    else:
        mode = "f32r"
    if merged_first and not mode.startswith("mgd"):
        ext, pad_idx, caps, blk_of, cmax, u8_ok = _prep(
            inputs["values"], inputs["lengths"], inputs["kings"],
            inputs["factorization_mask"], merged=False)
    p = _mode_params(mode)
    nchk = p["nchk"]
    cm_np = np.dtype(mybir.dt.np(p["cdt"]))
    out_np = np.dtype(mybir.dt.np(p["odt"]))
    tdt_np = np.dtype(mybir.dt.np(p["tdt"] if p["tdt"] != mybir.dt.float32r
                                  else mybir.dt.float32))

    nch = sum(caps)
    key = (caps, mode)
    if key not in _prog_cache:
        _prog_cache[key] = _build_program(caps, mode)
    nc = _prog_cache[key]

    if mode.startswith("mgd"):
        tabs = _make_tab_merged(inputs["pieces"], inputs["ranks"],
                                inputs["files"], inputs["tiles"],
                                inputs["factorization_mask"], blk_of, tdt_np)
    else:
        tabs = _make_tab_fact(inputs["pieces"], inputs["ranks"],
                              inputs["files"], inputs["tiles"], blk_of, mode)

    in_maps = []
    for c in range(N_CORES):
        sel = ext[pad_idx[c]]                  # [nch*128, nfp] f32
        cmh = sel.reshape(nch, 128, nchk, 128).transpose(3, 0, 2, 1)
        if mode == "mgd4":
            # nibble-pack: byte = count[f] | count[f+384] << 4
            cmh = cmh[:, :, :3, :] + 16.0 * cmh[:, :, 3:, :]
        in_maps.append({
            "tab": tabs[c],
            "cm": np.ascontiguousarray(cmh.reshape(128, -1).astype(cm_np)),
        })

    res = run_bass_kernel_spmd(nc, in_maps, list(range(N_CORES)),
                               trace=trace)

    comb = np.zeros((2 * B, DOUT), np.float32)
    for c in range(N_CORES):
        # out dram layout is [128, nch*DOUT] partition-major; the device
        # stores raw sums — the clip to [0, 1] happens here on the host
        flat = (res.results[c]["out"].astype(np.float32)
                .reshape(128, nch, DOUT).transpose(1, 0, 2)
                .reshape(nch * 128, DOUT))
        valid = pad_idx[c] >= 0
        comb[pad_idx[c][valid]] = flat[valid]
    np.clip(comb, 0.0, 1.0, out=comb)
    return (comb[:B], comb[B:]), res


def kernel(**inputs):
    (a, b), _ = _run(inputs, trace=False)
    return a, b


# revision 44
# speedup vs baseline: 1.1705x; 1.0317x over previous
"""Trainium2 kernel for the NNUE-style factorized embedding segment-sum.

Strategy: the ragged two-table embedding-bag is reformulated as block-diagonal
dense matmuls.  For each output row (bag), the gather+segment-sum over its
ragged feature ids equals  counts_row @ table_block, where table_block is the
768-row slice of the merged factorized table selected by the bag's king square
(and counts columns are flip-remapped for the second output so only ONE table
is ever needed).

Host (integer work only): merge the factor tables (tiles + (pieces+ranks+
files)*mask -> [64, 768, 256]), build per-bag count rows, group (output,bag)
items by table block, shard blocks over 8 cores, clip outputs.  Device (fp
work): per 128-item chunk, 6 accumulating matmuls (K=128, M=128, N=256) and
a PSUM->fp16 drain.

Default mode "mgd8": merged table in fp16, counts as fp8e4 (ints <= 16 exact,
consumed by the matmul directly, no on-device cast), outputs in fp16
(upcast + clip on host).  Fallbacks: "mgdu8" (uint8 counts + on-device cast)
if counts exceed 16, and the original factorized "hilo"/"f32r" paths.

Scheduling (from NTFF profiling; 82us -> ~43us): table loads ride the ACT
HWDGE ring (first two up front split fine, the rest two slots ahead so they
do not starve count loads of shared SDMA bandwidth during fill), count loads
ride the SP ring, batched per-slot stores ride the ACT ring behind all table
loads; dummy matmuls on memset tiles warm the PE p-state (0.65->2.4 GHz over
~3us) during the DMA fill; the final store is kept small because its HBM
write receipt gates the exit barrier.

Blocks are assigned to (core, slot) so that each slot's chunk capacity (shared
across cores — the compiled program is SPMD) matches the data tightly; for
this input the packing is optimal (36 chunks/core vs 35.875 ideal).
"""

import numpy as np
import ml_dtypes

import concourse.bass as bass
import concourse.tile as tile
from concourse import bacc, mybir
from concourse.bass_utils import run_bass_kernel_spmd

N_CORES = 8
B = 16384          # bags
KPL = 12           # piece planes
DOUT = 256
PIECE = 768        # KPL * 64
NFEAT = 972        # 768 tiles + 12 pieces + 96 ranks + 96 files (factorized)
NBLK = 8           # table blocks per core (64 king squares / 8 cores)

# ---------------------------------------------------------------------------
# host-side integer prep tables
_sq = np.arange(64)
_PERM = (7 - _sq // 8) * 8 + _sq % 8          # vertical king-square flip
_v = np.arange(PIECE)
_vk, _vr, _vf = _v // 64, (_v % 64) // 8, _v % 8
_FLIP_COL = ((_vk + 6) % 12) * 64 + (7 - _vr) * 8 + _vf

_prog_cache = {}


def _mode_params(mode):
    f32 = mybir.dt.float32
    if mode == "mgd4":
        # merged fp16 table, nibble-packed counts (two 4-bit counts per
        # byte) unpacked on DVE/Pool, fp16 out (clip on host)
        return dict(nchk=6, npass=1, tdt=mybir.dt.float16,
                    cdt=mybir.dt.uint8, mdt=mybir.dt.float16,
                    odt=mybir.dt.float16, ccols=384)
    if mode == "mgd8":
        # merged fp16 table, fp8e4 counts straight into the matmul, fp16 out
        return dict(nchk=6, npass=1, tdt=mybir.dt.float16,
                    cdt=mybir.dt.float8e4, mdt=mybir.dt.float8e4,
                    odt=mybir.dt.float16, ccols=768)
    if mode == "mgdu8":
        return dict(nchk=6, npass=1, tdt=mybir.dt.float16,
                    cdt=mybir.dt.uint8, mdt=mybir.dt.float16,
                    odt=mybir.dt.float16, ccols=768)
    if mode == "hilo":
        return dict(nchk=8, npass=2, tdt=mybir.dt.bfloat16,
                    cdt=mybir.dt.uint8, mdt=mybir.dt.bfloat16, odt=f32,
                    ccols=1024)
    # f32r: factorized, fp32 tables with reduced-precision matmul
    return dict(nchk=8, npass=1, tdt=mybir.dt.float32r,
                cdt=mybir.dt.float32r, mdt=mybir.dt.float32r, odt=f32,
                ccols=1024)


def _build_program(caps: tuple, mode: str):
    """Bass program for one core.

    caps[s] = number of 128-item chunks for block slot s (shared by all
    cores).  Per slot: DMA table block + counts, (maybe) cast counts, then per
    chunk npass*nchk accumulating matmuls and a clipped PSUM->SBUF->HBM drain.
    """
    p = _mode_params(mode)
    nchk, npass = p["nchk"], p["npass"]
    tdt, cdt, mdt, odt = p["tdt"], p["cdt"], p["mdt"], p["odt"]
    ccols = p["ccols"]
    nib = mode == "mgd4"
    cast = cdt != mdt and not nib

    nch = sum(caps)
    nc = bacc.Bacc("TRN2", target_bir_lowering=False, debug=False)
    f32 = mybir.dt.float32

    tabw = npass * nchk * DOUT
    # tab[p, blk*tabw + (pass*nchk+j)*DOUT + d] = table[blk,pass][j*128+p, d]
    tab = nc.dram_tensor("tab", [128, NBLK * tabw], tdt,
                         kind="ExternalInput").ap()
    # cm[p, (chunkbase(s)+i)*ccols + j*128 + m]
    #    = counts^T[slot s, chunk i][feature j*128+p, item m]
    # (mgd4: byte packs features f and f+384: lo nibble f, hi nibble f+384)
    cm = nc.dram_tensor("cm", [128, nch * ccols], cdt,
                        kind="ExternalInput").ap()
    # out[p, (chunkbase(s)+i)*DOUT + d]: partition-major so each per-slot
    # store is one DMA with caps*512B contiguous per partition line
    out = nc.dram_tensor("out", [128, nch * DOUT], odt,
                         kind="ExternalOutput").ap()

    cbase = np.concatenate([[0], np.cumsum(caps)]).astype(int)
    maxw = max(caps) * ccols

    with tile.TileContext(nc) as tc:
        with (
            tc.tile_pool(name="tabp", bufs=NBLK) as tabp,
            tc.tile_pool(name="cmup", bufs=8) as cmup,
            tc.tile_pool(name="cmp", bufs=5) as cmp_,
            tc.tile_pool(name="outp", bufs=8) as outp,
            tc.tile_pool(name="warmp", bufs=1) as wmp,
            tc.tile_pool(name="ps", bufs=8, space="PSUM") as psp,
        ):
            # PE p-state warmup: the tensor engine ramps 0.65->1.2->2.4 GHz
            # over ~3us of continuous execution.  Run dummy matmuls on
            # memset tiles during the DMA fill window so the real matmul
            # stream starts at full clock.
            wl = wmp.tile([128, 128], mdt, tag="warml")
            wr = wmp.tile([128, DOUT], tdt, tag="warmr")
            nc.gpsimd.memset(wl[:], 0)
            nc.gpsimd.memset(wr[:], 0)
            wp = psp.tile([128, DOUT], f32, tag="ps")
            for _ in range(10):
                nc.tensor.matmul(wp[:], lhsT=wl[:], rhs=wr[:])

            # table loads go on the ACT HWDGE ring (stores are emitted
            # later, so prefetches never block behind them).  Only the
            # first two tables load up front: both rings share the 16 SDMA
            # engines round-robin, so eagerly loading all tables would
            # halve the bandwidth available to the count loads during the
            # pipeline fill.  The rest are issued two slots ahead, batched
            # in PAIRS (786KB per DMA — bigger transfers run ~8% closer to
            # line rate and cost half the ring issue slots).
            def load_tab(b, tsplit=1, nslots=1):
                tt = tabp.tile([128, nslots * tabw], tdt, tag="tab")
                w = nslots * tabw
                tb = [w * k // tsplit // DOUT * DOUT
                      for k in range(tsplit + 1)]
                for k in range(tsplit):
                    nc.scalar.dma_start(
                        tt[:, tb[k]:tb[k + 1]],
                        tab[:, b * tabw + tb[k]:b * tabw + tb[k + 1]])
                return tt

            # tts[b] -> (tile, column offset of slot b's table within it)
            tts = [(load_tab(0, tsplit=3), 0), (load_tab(1), 0)]
            for b in range(NBLK):
                cmw = caps[b] * ccols
                c0 = cbase[b] * ccols
                # split ranges: slot 0 goes [chunk0, chunk1, rest] — the
                # first two matmul chunks get their own completion sems so
                # the stream starts early, without paying per-chunk issue
                # cost (~650ns each) that would delay the cm loads of the
                # following slots.  Whole-slot afterwards.
                if b == 0:
                    bnds = sorted(set([0, ccols, min(2 * ccols, cmw), cmw]))
                    nsplit = len(bnds) - 1
                else:
                    bnds = [0, cmw]
                    nsplit = 1
                tt, toff = tts[b]
                cu = cmup.tile([128, maxw], cdt, tag="cmu")
                for k in range(nsplit):
                    nc.sync.dma_start(
                        cu[:, bnds[k]:bnds[k + 1]],
                        cm[:, c0 + bnds[k]:c0 + bnds[k + 1]])
                nb = b + 2
                while len(tts) < NBLK and len(tts) <= nb + 1:
                    # prefetch a pair of tables (or a final singleton)
                    first = len(tts)
                    ns = min(2, NBLK - first)
                    pt = load_tab(first, nslots=ns)
                    for q in range(ns):
                        tts.append((pt, q * tabw))
                if nib:
                    # unpack nibbles: lo -> feature chunks 0..2, hi -> 3..5;
                    # lo on DVE, hi on Pool (gpsimd)
                    h0 = caps[b] * 384
                    cmt = cmp_.tile([128, 2 * maxw], mdt, tag="cm")
                    for k in range(nsplit):
                        nc.vector.tensor_scalar(
                            cmt[:, bnds[k]:bnds[k + 1]],
                            cu[:, bnds[k]:bnds[k + 1]],
                            15, None, mybir.AluOpType.bitwise_and)
                        nc.gpsimd.tensor_scalar(
                            cmt[:, h0 + bnds[k]:h0 + bnds[k + 1]],
                            cu[:, bnds[k]:bnds[k + 1]],
                            4, None, mybir.AluOpType.logical_shift_right)
                elif cast:
                    cmt = cmp_.tile([128, maxw], mdt, tag="cm")
                    # 8-bit -> 16-bit cast, split so it pipelines; alternate
                    # DVE / Pool so neither engine becomes the bottleneck
                    ncast = max(nsplit, 2)
                    cbnds = [cmw * k // ncast // 128 * 128
                             for k in range(ncast + 1)]
                    for k in range(ncast):
                        eng = nc.vector if k % 2 == 0 else nc.gpsimd
                        eng.tensor_copy(cmt[:, cbnds[k]:cbnds[k + 1]],
                                        cu[:, cbnds[k]:cbnds[k + 1]])
                else:
                    cmt = cu

                outt = outp.tile([128, caps[b] * DOUT], odt, tag="out")
                for i in range(caps[b]):
                    ps = psp.tile([128, DOUT], f32, tag="ps")
                    nmm = npass * nchk
                    for q in range(nmm):
                        p_, j = divmod(q, nchk)
                        if nib:
                            cb_ = (j // 3) * h0 + (i * 3 + j % 3) * 128
                        else:
                            cb_ = (i * nchk + j) * 128
                        lhsT = cmt[:, cb_:cb_ + 128]
                        nc.tensor.matmul(
                            ps[:],
                            lhsT=lhsT,
                            rhs=tt[:, toff + (p_ * nchk + j) * DOUT:
                                   toff + (p_ * nchk + j + 1) * DOUT],
                            start=(q == 0),
                            stop=(q == nmm - 1),
                        )
                    # clip(psum, 0, 1) -> per-slot sbuf tile (per chunk)
                    nc.vector.tensor_scalar(
                        outt[:, i * DOUT:(i + 1) * DOUT], ps[:],
                        1.0, 0.0, mybir.AluOpType.min, mybir.AluOpType.max)
                if b < NBLK - 1:
                    # one batched store per slot on the ACT HWDGE ring
                    nc.scalar.dma_start(
                        out[:, cbase[b] * DOUT:(cbase[b] + caps[b]) * DOUT],
                        outt[:])
                else:
                    # last slot: all-but-last chunks in one store, then the
                    # final chunk alone so the last HBM write receipt (which
                    # gates the exit barrier) covers a small transfer
                    if caps[b] > 1:
                        nc.scalar.dma_start(
                            out[:, cbase[b] * DOUT:
                                (cbase[b] + caps[b] - 1) * DOUT],
                            outt[:, :(caps[b] - 1) * DOUT])
                    nc.scalar.dma_start(
                        out[:, (cbase[b] + caps[b] - 1) * DOUT:
                            (cbase[b] + caps[b]) * DOUT],
                        outt[:, (caps[b] - 1) * DOUT:])

    nc.compile()
    return nc


def _prep(values, lengths, kings, mask, merged):
    """Host prep: counts, per-core item layout; factor sums if not merged."""
    values = np.asarray(values).astype(np.int64)
    lengths = np.asarray(lengths).astype(np.int64)
    kings = np.asarray(kings).astype(np.int64)
    maskrows = np.asarray(mask, np.float32).reshape(64, PIECE)

    seg = np.repeat(np.arange(B, dtype=np.int64), lengths)

    # counts in merged-table column space; output b columns are flip-remapped
    cnt_a = np.bincount(seg * PIECE + values,
                        minlength=B * PIECE).reshape(B, PIECE)
    cnt_b = np.bincount(seg * PIECE + _FLIP_COL[values],
                        minlength=B * PIECE).reshape(B, PIECE)

    # block id per (output,bag) item, in merged-table space
    blk = np.concatenate([kings[:, 0], _PERM[kings[:, 1]]])

    nfp = PIECE if merged else 1024
    ext = np.zeros((2 * B + 1, nfp), np.float32)  # last row stays zero (pad)
    cnt = ext[:2 * B, :PIECE]
    cnt[:B] = cnt_a
    cnt[B:] = cnt_b
    cmax = float(cnt.max())
    if not merged:
        # factorized extension: mask-weighted per-(k), (k,rank), (k,file) sums
        m = (cnt * maskrows[blk]).reshape(2 * B, KPL, 8, 8)
        ext[:2 * B, PIECE:PIECE + KPL] = m.sum(axis=(2, 3))
        ext[:2 * B, PIECE + KPL:PIECE + KPL + 96] = \
            m.sum(axis=3).reshape(2 * B, 96)
        ext[:2 * B, PIECE + KPL + 96:NFEAT] = \
            m.sum(axis=2).reshape(2 * B, 96)

    order = np.argsort(blk, kind="stable")
    nper = np.bincount(blk, minlength=64)
    offs = np.concatenate([[0], np.cumsum(nper)])
    nchunks = np.maximum(np.ceil(nper / 128).astype(int), 1)

    # assign blocks to (core, slot): sort by descending chunk need so each
    # slot's shared capacity is tight
    rank = np.argsort(-nchunks, kind="stable")      # block ids, desc need
    caps = tuple(int(nchunks[rank[s * N_CORES]]) for s in range(NBLK))
    cbase = np.concatenate([[0], np.cumsum(caps)]).astype(int)
    nch = int(cbase[-1])

    pad_idx = np.full((N_CORES, nch * 128), -1, np.int64)
    for s in range(NBLK):
        for c in range(N_CORES):
            t = rank[s * N_CORES + c]               # block for (core c, slot s)
            ids = order[offs[t]:offs[t + 1]]
            base = cbase[s] * 128
            pad_idx[c, base:base + len(ids)] = ids

    # block index (0..63) per (core, slot), for table selection
    blk_of = rank.reshape(NBLK, N_CORES).T          # [core, slot]

    u8_ok = (np.all((maskrows == 0.0) | (maskrows == 1.0))
             and ext.max() <= 255.0)
    return ext, pad_idx, caps, blk_of, cmax, u8_ok


def _make_tab_merged(pieces, ranks, files, tiles, mask, blk_of, tdt_np):
    """Per-core [128, NBLK*6*DOUT]: host-merged factorized table."""
    p = np.asarray(pieces, np.float32)   # [64,12,1,1,256]
    r = np.asarray(ranks, np.float32)    # [64,12,8,1,256]
    f = np.asarray(files, np.float32)    # [64,12,1,8,256]
    t = np.asarray(tiles, np.float32)    # [64,12,8,8,256]
    m = np.asarray(mask, np.float32)     # [64,12,8,8,1]
    merged = (t + (p + r + f) * m).reshape(64, PIECE, DOUT).astype(tdt_np)
    planes = merged.reshape(64, 6, 128, DOUT)
    tabs = []
    for c in range(N_CORES):
        tc_ = planes[blk_of[c]]                # [8, 6, 128, 256]
        tabs.append(np.ascontiguousarray(
            tc_.transpose(2, 0, 1, 3).reshape(128, -1)))
    return tabs


def _make_tab_fact(pieces, ranks, files, tiles, blk_of, mode):
    """Per-core factorized tables (hilo / f32r fallback paths)."""
    pieces = np.asarray(pieces, np.float32).reshape(64, KPL, DOUT)
    ranks = np.asarray(ranks, np.float32).reshape(64, KPL * 8, DOUT)
    files = np.asarray(files, np.float32).reshape(64, KPL * 8, DOUT)
    tiles = np.asarray(tiles, np.float32).reshape(64, PIECE, DOUT)
    big = np.zeros((64, 1024, DOUT), np.float32)
    big[:, :PIECE] = tiles
    big[:, PIECE:PIECE + KPL] = pieces
    big[:, PIECE + KPL:PIECE + KPL + 96] = ranks
    big[:, PIECE + KPL + 96:NFEAT] = files

    bf16 = ml_dtypes.bfloat16
    if mode == "hilo":
        hi = big.astype(bf16)
        lo = (big - hi.astype(np.float32)).astype(bf16)
        planes = np.stack([hi, lo], axis=1).reshape(64, 2, 8, 128, DOUT)
    else:
        planes = big.reshape(64, 1, 8, 128, DOUT)

    tabs = []
    for c in range(N_CORES):
        t = planes[blk_of[c]]                  # [8, npass, 8, 128, DOUT]
        t = t.transpose(3, 0, 1, 2, 4)         # [128, slot, pass, chunk, dout]
        tabs.append(np.ascontiguousarray(t.reshape(128, -1)))
    return tabs


def _run(inputs, trace=False, force_mode=None):
    merged_first = force_mode is None or force_mode.startswith("mgd")
    ext, pad_idx, caps, blk_of, cmax, u8_ok = _prep(
        inputs["values"], inputs["lengths"], inputs["kings"],
        inputs["factorization_mask"], merged=merged_first)
    if force_mode:
        mode = force_mode
    elif cmax <= 16.0:       # ints <= 16 are exact in fp8 e4m3
        mode = "mgd8"
    elif cmax <= 255.0:
        mode = "mgdu8"
    else:
        mode = "f32r"
    if merged_first and not mode.startswith("mgd"):
        ext, pad_idx, caps, blk_of, cmax, u8_ok = _prep(
            inputs["values"], inputs["lengths"], inputs["kings"],
            inputs["factorization_mask"], merged=False)
    p = _mode_params(mode)
    nchk = p["nchk"]
    cm_np = np.dtype(mybir.dt.np(p["cdt"]))
    out_np = np.dtype(mybir.dt.np(p["odt"]))
    tdt_np = np.dtype(mybir.dt.np(p["tdt"] if p["tdt"] != mybir.dt.float32r
                                  else mybir.dt.float32))

    nch = sum(caps)
    key = (caps, mode)
    if key not in _prog_cache:
        _prog_cache[key] = _build_program(caps, mode)
    nc = _prog_cache[key]

    if mode.startswith("mgd"):
        tabs = _make_tab_merged(inputs["pieces"], inputs["ranks"],
                                inputs["files"], inputs["tiles"],
                                inputs["factorization_mask"], blk_of, tdt_np)
    else:
        tabs = _make_tab_fact(inputs["pieces"], inputs["ranks"],
                              inputs["files"], inputs["tiles"], blk_of, mode)

    in_maps = []
    for c in range(N_CORES):
        sel = ext[pad_idx[c]]                  # [nch*128, nfp] f32
        cmh = sel.reshape(nch, 128, nchk, 128).transpose(3, 0, 2, 1)
        if mode == "mgd4":
            # nibble-pack: byte = count[f] | count[f+384] << 4
            cmh = cmh[:, :, :3, :] + 16.0 * cmh[:, :, 3:, :]
        in_maps.append({
            "tab": tabs[c],
            "cm": np.ascontiguousarray(cmh.reshape(128, -1).astype(cm_np)),
        })

    res = run_bass_kernel_spmd(nc, in_maps, list(range(N_CORES)),
                               trace=trace)

    comb = np.zeros((2 * B, DOUT), np.float32)
    for c in range(N_CORES):
        # out dram layout is [128, nch*DOUT] partition-major; the device
        # stores raw sums — the clip to [0, 1] happens here on the host
        flat = (res.results[c]["out"].astype(np.float32)
                .reshape(128, nch, DOUT).transpose(1, 0, 2)
                .reshape(nch * 128, DOUT))
        valid = pad_idx[c] >= 0
        comb[pad_idx[c][valid]] = flat[valid]
    np.clip(comb, 0.0, 1.0, out=comb)
    return (comb[:B], comb[B:]), res


def kernel(**inputs):
    (a, b), _ = _run(inputs, trace=False)
    return a, b


# revision 46
# speedup vs baseline: 1.1877x; 1.0147x over previous
"""Trainium2 kernel for the NNUE-style factorized embedding segment-sum.

Strategy: the ragged two-table embedding-bag is reformulated as block-diagonal
dense matmuls.  For each output row (bag), the gather+segment-sum over its
ragged feature ids equals  counts_row @ table_block, where table_block is the
768-row slice of the merged factorized table selected by the bag's king square
(and counts columns are flip-remapped for the second output so only ONE table
is ever needed).

Host (integer work only): merge the factor tables (tiles + (pieces+ranks+
files)*mask -> [64, 768, 256]), build per-bag count rows, group (output,bag)
items by table block, shard blocks over 8 cores, clip outputs.  Device (fp
work): per 128-item chunk, 6 accumulating matmuls (K=128, M=128, N=256) and
a PSUM->fp16 drain.

Default mode "mgd8": merged table in fp16, counts as fp8e4 (ints <= 16 exact,
consumed by the matmul directly, no on-device cast), outputs in fp16
(upcast + clip on host).  Fallbacks: "mgdu8" (uint8 counts + on-device cast)
if counts exceed 16, and the original factorized "hilo"/"f32r" paths.

Scheduling (from NTFF profiling; 82us -> ~43us): table loads ride the ACT
HWDGE ring (first two up front split fine, the rest two slots ahead so they
do not starve count loads of shared SDMA bandwidth during fill), count loads
ride the SP ring, batched per-slot stores ride the ACT ring behind all table
loads; dummy matmuls on memset tiles warm the PE p-state (0.65->2.4 GHz over
~3us) during the DMA fill; the final store is kept small because its HBM
write receipt gates the exit barrier.

Blocks are assigned to (core, slot) so that each slot's chunk capacity (shared
across cores — the compiled program is SPMD) matches the data tightly; for
this input the packing is optimal (36 chunks/core vs 35.875 ideal).
"""

import numpy as np
import ml_dtypes

import concourse.bass as bass
import concourse.tile as tile
from concourse import bacc, mybir
from concourse.bass_utils import run_bass_kernel_spmd

N_CORES = 8
B = 16384          # bags
KPL = 12           # piece planes
DOUT = 256
PIECE = 768        # KPL * 64
NFEAT = 972        # 768 tiles + 12 pieces + 96 ranks + 96 files (factorized)
NBLK = 8           # table blocks per core (64 king squares / 8 cores)

# ---------------------------------------------------------------------------
# host-side integer prep tables
_sq = np.arange(64)
_PERM = (7 - _sq // 8) * 8 + _sq % 8          # vertical king-square flip
_v = np.arange(PIECE)
_vk, _vr, _vf = _v // 64, (_v % 64) // 8, _v % 8
_FLIP_COL = ((_vk + 6) % 12) * 64 + (7 - _vr) * 8 + _vf

_prog_cache = {}


def _mode_params(mode):
    f32 = mybir.dt.float32
    if mode == "mgd4":
        # merged fp16 table, nibble-packed counts (two 4-bit counts per
        # byte) unpacked on DVE/Pool, fp16 out (clip on host)
        return dict(nchk=6, npass=1, tdt=mybir.dt.float16,
                    cdt=mybir.dt.uint8, mdt=mybir.dt.float16,
                    odt=mybir.dt.float16, ccols=384)
    if mode == "mgd8":
        # merged fp16 table, fp8e4 counts straight into the matmul, fp16 out
        return dict(nchk=6, npass=1, tdt=mybir.dt.float16,
                    cdt=mybir.dt.float8e4, mdt=mybir.dt.float8e4,
                    odt=mybir.dt.float16, ccols=768)
    if mode == "mgdu8":
        return dict(nchk=6, npass=1, tdt=mybir.dt.float16,
                    cdt=mybir.dt.uint8, mdt=mybir.dt.float16,
                    odt=mybir.dt.float16, ccols=768)
    if mode == "hilo":
        return dict(nchk=8, npass=2, tdt=mybir.dt.bfloat16,
                    cdt=mybir.dt.uint8, mdt=mybir.dt.bfloat16, odt=f32,
                    ccols=1024)
    # f32r: factorized, fp32 tables with reduced-precision matmul
    return dict(nchk=8, npass=1, tdt=mybir.dt.float32r,
                cdt=mybir.dt.float32r, mdt=mybir.dt.float32r, odt=f32,
                ccols=1024)


def _build_program(caps: tuple, mode: str):
    """Bass program for one core.

    caps[s] = number of 128-item chunks for block slot s (shared by all
    cores).  Per slot: DMA table block + counts, (maybe) cast counts, then per
    chunk npass*nchk accumulating matmuls and a clipped PSUM->SBUF->HBM drain.
    """
    p = _mode_params(mode)
    nchk, npass = p["nchk"], p["npass"]
    tdt, cdt, mdt, odt = p["tdt"], p["cdt"], p["mdt"], p["odt"]
    ccols = p["ccols"]
    nib = mode == "mgd4"
    cast = cdt != mdt and not nib

    nch = sum(caps)
    nc = bacc.Bacc("TRN2", target_bir_lowering=False, debug=False)
    f32 = mybir.dt.float32

    tabw = npass * nchk * DOUT
    # tab[p, blk*tabw + (pass*nchk+j)*DOUT + d] = table[blk,pass][j*128+p, d]
    tab = nc.dram_tensor("tab", [128, NBLK * tabw], tdt,
                         kind="ExternalInput").ap()
    # cm[p, (chunkbase(s)+i)*ccols + j*128 + m]
    #    = counts^T[slot s, chunk i][feature j*128+p, item m]
    # (mgd4: byte packs features f and f+384: lo nibble f, hi nibble f+384)
    cm = nc.dram_tensor("cm", [128, nch * ccols], cdt,
                        kind="ExternalInput").ap()
    # out[p, (chunkbase(s)+i)*DOUT + d]: partition-major so each per-slot
    # store is one DMA with caps*512B contiguous per partition line
    out = nc.dram_tensor("out", [128, nch * DOUT], odt,
                         kind="ExternalOutput").ap()

    cbase = np.concatenate([[0], np.cumsum(caps)]).astype(int)
    maxw = max(caps) * ccols

    with tile.TileContext(nc) as tc:
        with (
            tc.tile_pool(name="tabp", bufs=NBLK) as tabp,
            tc.tile_pool(name="cmup", bufs=8) as cmup,
            tc.tile_pool(name="cmp", bufs=5) as cmp_,
            tc.tile_pool(name="outp", bufs=8) as outp,
            tc.tile_pool(name="warmp", bufs=1) as wmp,
            tc.tile_pool(name="ps", bufs=8, space="PSUM") as psp,
        ):
            # PE p-state warmup: the tensor engine ramps 0.65->1.2->2.4 GHz
            # over ~3us of continuous execution.  Run dummy matmuls on
            # memset tiles during the DMA fill window so the real matmul
            # stream starts at full clock.
            wl = wmp.tile([128, 128], mdt, tag="warml")
            wr = wmp.tile([128, DOUT], tdt, tag="warmr")
            nc.gpsimd.memset(wl[:], 0)
            nc.gpsimd.memset(wr[:], 0)
            wp = psp.tile([128, DOUT], f32, tag="ps")
            for _ in range(10):
                nc.tensor.matmul(wp[:], lhsT=wl[:], rhs=wr[:])

            # table loads go on the ACT HWDGE ring (stores are emitted
            # later, so prefetches never block behind them).  Only the
            # first two tables load up front: both rings share the 16 SDMA
            # engines round-robin, so eagerly loading all tables would
            # halve the bandwidth available to the count loads during the
            # pipeline fill.  The rest are issued two slots ahead, batched
            # in PAIRS (786KB per DMA — bigger transfers run ~8% closer to
            # line rate and cost half the ring issue slots).
            def load_tab(b, tsplit=1, nslots=1):
                tt = tabp.tile([128, nslots * tabw], tdt, tag="tab")
                w = nslots * tabw
                tb = [w * k // tsplit // DOUT * DOUT
                      for k in range(tsplit + 1)]
                for k in range(tsplit):
                    nc.scalar.dma_start(
                        tt[:, tb[k]:tb[k + 1]],
                        tab[:, b * tabw + tb[k]:b * tabw + tb[k + 1]])
                return tt

            # tts[b] -> (tile, column offset of slot b's table within it)
            tts = [(load_tab(0, tsplit=3), 0), (load_tab(1), 0)]
            cus = {}
            for b in range(NBLK):
                cmw = caps[b] * ccols
                c0 = cbase[b] * ccols
                # split ranges: slot 0 goes [chunk0, chunk1, rest] — the
                # first two matmul chunks get their own completion sems so
                # the stream starts early, without paying per-chunk issue
                # cost (~650ns each) that would delay the cm loads of the
                # following slots.  Whole-slot afterwards.
                if b == 0:
                    bnds = sorted(set([0, ccols, min(2 * ccols, cmw), cmw]))
                    nsplit = len(bnds) - 1
                else:
                    bnds = [0, cmw]
                    nsplit = 1
                tt, toff = tts[b]
                if b <= 2 or cast or nib:
                    # fill-critical slots (and fallback modes): own tile
                    cu = cmup.tile([128, maxw], cdt, tag="cmu")
                    coff = 0
                    cus[b] = (cu, 0)
                    for k in range(nsplit):
                        nc.sync.dma_start(
                            cu[:, bnds[k]:bnds[k + 1]],
                            cm[:, c0 + bnds[k]:c0 + bnds[k + 1]])
                elif b in cus:
                    cu, coff = cus[b]          # loaded with the previous slot
                else:
                    # steady phase: batch this slot and the next into one
                    # DMA (bigger transfer runs closer to line rate, one
                    # ring issue slot instead of two)
                    ns = min(2, NBLK - b)
                    w = sum(caps[b:b + ns]) * ccols
                    cu = cmup.tile([128, w], cdt, tag="cmu")
                    nc.sync.dma_start(cu[:, 0:w], cm[:, c0:c0 + w])
                    off = 0
                    for q in range(ns):
                        cus[b + q] = (cu, off)
                        off += caps[b + q] * ccols
                    coff = 0
                nb = b + 2
                while len(tts) < NBLK and len(tts) <= nb + 1:
                    # prefetch a pair of tables (or a final singleton)
                    first = len(tts)
                    ns = min(2, NBLK - first)
                    pt = load_tab(first, nslots=ns)
                    for q in range(ns):
                        tts.append((pt, q * tabw))
                if nib:
                    # unpack nibbles: lo -> feature chunks 0..2, hi -> 3..5;
                    # lo on DVE, hi on Pool (gpsimd)
                    h0 = caps[b] * 384
                    cmt = cmp_.tile([128, 2 * maxw], mdt, tag="cm")
                    for k in range(nsplit):
                        nc.vector.tensor_scalar(
                            cmt[:, bnds[k]:bnds[k + 1]],
                            cu[:, bnds[k]:bnds[k + 1]],
                            15, None, mybir.AluOpType.bitwise_and)
                        nc.gpsimd.tensor_scalar(
                            cmt[:, h0 + bnds[k]:h0 + bnds[k + 1]],
                            cu[:, bnds[k]:bnds[k + 1]],
                            4, None, mybir.AluOpType.logical_shift_right)
                elif cast:
                    cmt = cmp_.tile([128, maxw], mdt, tag="cm")
                    # 8-bit -> 16-bit cast, split so it pipelines; alternate
                    # DVE / Pool so neither engine becomes the bottleneck
                    ncast = max(nsplit, 2)
                    cbnds = [cmw * k // ncast // 128 * 128
                             for k in range(ncast + 1)]
                    for k in range(ncast):
                        eng = nc.vector if k % 2 == 0 else nc.gpsimd
                        eng.tensor_copy(cmt[:, cbnds[k]:cbnds[k + 1]],
                                        cu[:, cbnds[k]:cbnds[k + 1]])
                else:
                    cmt = cu

                outt = outp.tile([128, caps[b] * DOUT], odt, tag="out")
                for i in range(caps[b]):
                    ps = psp.tile([128, DOUT], f32, tag="ps")
                    nmm = npass * nchk
                    for q in range(nmm):
                        p_, j = divmod(q, nchk)
                        if nib:
                            cb_ = (j // 3) * h0 + (i * 3 + j % 3) * 128
                        else:
                            cb_ = coff + (i * nchk + j) * 128
                        lhsT = cmt[:, cb_:cb_ + 128]
                        nc.tensor.matmul(
                            ps[:],
                            lhsT=lhsT,
                            rhs=tt[:, toff + (p_ * nchk + j) * DOUT:
                                   toff + (p_ * nchk + j + 1) * DOUT],
                            start=(q == 0),
                            stop=(q == nmm - 1),
                        )
                    # clip(psum, 0, 1) -> per-slot sbuf tile (per chunk)
                    nc.vector.tensor_scalar(
                        outt[:, i * DOUT:(i + 1) * DOUT], ps[:],
                        1.0, 0.0, mybir.AluOpType.min, mybir.AluOpType.max)
                if b < NBLK - 1:
                    # one batched store per slot on the ACT HWDGE ring
                    nc.scalar.dma_start(
                        out[:, cbase[b] * DOUT:(cbase[b] + caps[b]) * DOUT],
                        outt[:])
                else:
                    # last slot: all-but-last chunks in one store, then the
                    # final chunk alone so the last HBM write receipt (which
                    # gates the exit barrier) covers a small transfer
                    if caps[b] > 1:
                        nc.scalar.dma_start(
                            out[:, cbase[b] * DOUT:
                                (cbase[b] + caps[b] - 1) * DOUT],
                            outt[:, :(caps[b] - 1) * DOUT])
                    nc.scalar.dma_start(
                        out[:, (cbase[b] + caps[b] - 1) * DOUT:
                            (cbase[b] + caps[b]) * DOUT],
                        outt[:, (caps[b] - 1) * DOUT:])

    nc.compile()
    return nc


def _prep(values, lengths, kings, mask, merged):
    """Host prep: counts, per-core item layout; factor sums if not merged."""
    values = np.asarray(values).astype(np.int64)
    lengths = np.asarray(lengths).astype(np.int64)
    kings = np.asarray(kings).astype(np.int64)
    maskrows = np.asarray(mask, np.float32).reshape(64, PIECE)

    seg = np.repeat(np.arange(B, dtype=np.int64), lengths)

    # counts in merged-table column space; output b columns are flip-remapped
    cnt_a = np.bincount(seg * PIECE + values,
                        minlength=B * PIECE).reshape(B, PIECE)
    cnt_b = np.bincount(seg * PIECE + _FLIP_COL[values],
                        minlength=B * PIECE).reshape(B, PIECE)

    # block id per (output,bag) item, in merged-table space
    blk = np.concatenate([kings[:, 0], _PERM[kings[:, 1]]])

    nfp = PIECE if merged else 1024
    ext = np.zeros((2 * B + 1, nfp), np.float32)  # last row stays zero (pad)
    cnt = ext[:2 * B, :PIECE]
    cnt[:B] = cnt_a
    cnt[B:] = cnt_b
    cmax = float(cnt.max())
    if not merged:
        # factorized extension: mask-weighted per-(k), (k,rank), (k,file) sums
        m = (cnt * maskrows[blk]).reshape(2 * B, KPL, 8, 8)
        ext[:2 * B, PIECE:PIECE + KPL] = m.sum(axis=(2, 3))
        ext[:2 * B, PIECE + KPL:PIECE + KPL + 96] = \
            m.sum(axis=3).reshape(2 * B, 96)
        ext[:2 * B, PIECE + KPL + 96:NFEAT] = \
            m.sum(axis=2).reshape(2 * B, 96)

    order = np.argsort(blk, kind="stable")
    nper = np.bincount(blk, minlength=64)
    offs = np.concatenate([[0], np.cumsum(nper)])
    nchunks = np.maximum(np.ceil(nper / 128).astype(int), 1)

    # assign blocks to (core, slot): sort by descending chunk need so each
    # slot's shared capacity is tight
    rank = np.argsort(-nchunks, kind="stable")      # block ids, desc need
    caps = tuple(int(nchunks[rank[s * N_CORES]]) for s in range(NBLK))
    cbase = np.concatenate([[0], np.cumsum(caps)]).astype(int)
    nch = int(cbase[-1])

    pad_idx = np.full((N_CORES, nch * 128), -1, np.int64)
    for s in range(NBLK):
        for c in range(N_CORES):
            t = rank[s * N_CORES + c]               # block for (core c, slot s)
            ids = order[offs[t]:offs[t + 1]]
            base = cbase[s] * 128
            pad_idx[c, base:base + len(ids)] = ids

    # block index (0..63) per (core, slot), for table selection
    blk_of = rank.reshape(NBLK, N_CORES).T          # [core, slot]

    u8_ok = (np.all((maskrows == 0.0) | (maskrows == 1.0))
             and ext.max() <= 255.0)
    return ext, pad_idx, caps, blk_of, cmax, u8_ok


def _make_tab_merged(pieces, ranks, files, tiles, mask, blk_of, tdt_np):
    """Per-core [128, NBLK*6*DOUT]: host-merged factorized table."""
    p = np.asarray(pieces, np.float32)   # [64,12,1,1,256]
    r = np.asarray(ranks, np.float32)    # [64,12,8,1,256]
    f = np.asarray(files, np.float32)    # [64,12,1,8,256]
    t = np.asarray(tiles, np.float32)    # [64,12,8,8,256]
    m = np.asarray(mask, np.float32)     # [64,12,8,8,1]
    merged = (t + (p + r + f) * m).reshape(64, PIECE, DOUT).astype(tdt_np)
    planes = merged.reshape(64, 6, 128, DOUT)
    tabs = []
    for c in range(N_CORES):
        tc_ = planes[blk_of[c]]                # [8, 6, 128, 256]
        tabs.append(np.ascontiguousarray(
            tc_.transpose(2, 0, 1, 3).reshape(128, -1)))
    return tabs


def _make_tab_fact(pieces, ranks, files, tiles, blk_of, mode):
    """Per-core factorized tables (hilo / f32r fallback paths)."""
    pieces = np.asarray(pieces, np.float32).reshape(64, KPL, DOUT)
    ranks = np.asarray(ranks, np.float32).reshape(64, KPL * 8, DOUT)
    files = np.asarray(files, np.float32).reshape(64, KPL * 8, DOUT)
    tiles = np.asarray(tiles, np.float32).reshape(64, PIECE, DOUT)
    big = np.zeros((64, 1024, DOUT), np.float32)
    big[:, :PIECE] = tiles
    big[:, PIECE:PIECE + KPL] = pieces
    big[:, PIECE + KPL:PIECE + KPL + 96] = ranks
    big[:, PIECE + KPL + 96:NFEAT] = files

    bf16 = ml_dtypes.bfloat16
    if mode == "hilo":
        hi = big.astype(bf16)
        lo = (big - hi.astype(np.float32)).astype(bf16)
        planes = np.stack([hi, lo], axis=1).reshape(64, 2, 8, 128, DOUT)
    else:
        planes = big.reshape(64, 1, 8, 128, DOUT)

    tabs = []
    for c in range(N_CORES):
        t = planes[blk_of[c]]                  # [8, npass, 8, 128, DOUT]
        t = t.transpose(3, 0, 1, 2, 4)         # [128, slot, pass, chunk, dout]
        tabs.append(np.ascontiguousarray(t.reshape(128, -1)))
    return tabs


def _run(inputs, trace=False, force_mode=None):
    merged_first = force_mode is None or force_mode.startswith("mgd")
    ext, pad_idx, caps, blk_of, cmax, u8_ok = _prep(
        inputs["values"], inputs["lengths"], inputs["kings"],
        inputs["factorization_mask"], merged=merged_first)
    if force_mode:
        mode = force_mode
    elif cmax <= 16.0:       # ints <= 16 are exact in fp8 e4m3
        mode = "mgd8"
    elif cmax <= 255.0:
        mode = "mgdu8"
    else:
        mode = "f32r"
    if merged_first and not mode.startswith("mgd"):
        ext, pad_idx, caps, blk_of, cmax, u8_ok = _prep(
            inputs["values"], inputs["lengths"], inputs["kings"],
            inputs["factorization_mask"], merged=False)
    p = _mode_params(mode)
    nchk = p["nchk"]
    cm_np = np.dtype(mybir.dt.np(p["cdt"]))
    out_np = np.dtype(mybir.dt.np(p["odt"]))
    tdt_np = np.dtype(mybir.dt.np(p["tdt"] if p["tdt"] != mybir.dt.float32r
                                  else mybir.dt.float32))

    nch = sum(caps)
    key = (caps, mode)
    if key not in _prog_cache:
        _prog_cache[key] = _build_program(caps, mode)
    nc = _prog_cache[key]

    if mode.startswith("mgd"):
        tabs = _make_tab_merged(inputs["pieces"], inputs["ranks"],
                                inputs["files"], inputs["tiles"],
                                inputs["factorization_mask"], blk_of, tdt_np)
    else:
        tabs = _make_tab_fact(inputs["pieces"], inputs["ranks"],
                              inputs["files"], inputs["tiles"], blk_of, mode)

    in_maps = []
    for c in range(N_CORES):
        sel = ext[pad_idx[c]]                  # [nch*128, nfp] f32
        cmh = sel.reshape(nch, 128, nchk, 128).transpose(3, 0, 2, 1)
        if mode == "mgd4":
            # nibble-pack: byte = count[f] | count[f+384] << 4
            cmh = cmh[:, :, :3, :] + 16.0 * cmh[:, :, 3:, :]
        in_maps.append({
            "tab": tabs[c],
            "cm": np.ascontiguousarray(cmh.reshape(128, -1).astype(cm_np)),
        })

    res = run_bass_kernel_spmd(nc, in_maps, list(range(N_CORES)),
                               trace=trace)

    comb = np.zeros((2 * B, DOUT), np.float32)
    for c in range(N_CORES):
        # out dram layout is [128, nch*DOUT] partition-major; the device
        # stores raw sums — the clip to [0, 1] happens here on the host
        flat = (res.results[c]["out"].astype(np.float32)
                .reshape(128, nch, DOUT).transpose(1, 0, 2)
                .reshape(nch * 128, DOUT))
        valid = pad_idx[c] >= 0
        comb[pad_idx[c][valid]] = flat[valid]
    np.clip(comb, 0.0, 1.0, out=comb)
    return (comb[:B], comb[B:]), res


def kernel(**inputs):
    (a, b), _ = _run(inputs, trace=False)
    return a, b
